# revision 27
# baseline (speedup 1.0000x reference)
"""Multi-branch BatchNorm2d (16 branches sharing one batch-stat reduction).

Computation (reference):
    mean/var over (B,H,W) per channel of x[32,64,32,32], then for each of
    N=16 branches: out[:, n*64:(n+1)*64] = gamma[n,c]*xhat + beta[n,c],
    giving out[32, 1024, 32, 32].

Strategy (8 NeuronCores, branch-parallel, no collectives):
  - x is replicated: every core reads the full 8 MiB x and computes the
    (B,H,W) mean/var locally (the ncfw all-reduce costs 70-80 us/call on
    this setup - far more than the 20 us of extra read).
  - SBUF layout [128, 32, 512]: partition p = c*2 + h0 (h0 = H half),
    free (b, (h1 w)). x loads in ~1 MiB batch chunks on ONE HWDGE ring
    (uniform-size, single-ring DMAs empirically hold line rate; variable
    sizes or a second ring make the SDMA engines round-robin queues and
    skew chunk completions), with small first and last chunks so the
    stats pipeline starts early and drains right behind the final DMA.
  - Per chunk, two passes pipeline behind the DMA in parallel: DVE
    tensor_reduce(negate) accumulates -S per chunk (no junk output, no
    accumulator read), ACT accumulates raw Q via Square + accum.
  - A dummy Sqrt is the FIRST ACT instruction: the one Sqrt-table load
    (the `sqrt_and_others` set, which also contains Square and Identity)
    hoists to kernel start instead of landing between the last Square
    and the fold's Sqrt.
  - Fold: pair-combine (c,0)/(c,1) partitions via the 32-lane pair-swap
    stream_shuffle, then (nmean, ex2) = (-S, Q)/NTOT,
    -var = nmean^2 - ex2, sd = Sqrt(-var * -1 + eps), inv = 1/sd,
    A = gamma*inv, Bc = beta + nmean*A; out = x*A + Bc.
  - Each core computes N/8 = 2 branches: per group, the branch-0 tile
    comes from DVE (tensor_scalar) and the branch-1 tile from the
    otherwise-idle ACT engine (Identity with per-partition AP
    scale/bias), so the first two stores issue in parallel right after
    the fold and a DVE p-state throttle cannot stall the store stream.
    20 x ~1 MiB stores = 16 MiB of output writes per core, the HBM
    roofline for this memory-bound problem.
"""

import numpy as np

import concourse.bacc as bacc
import concourse.bass as bass
import concourse.tile as tile
from concourse import mybir
from concourse.bass_utils import run_bass_kernel_spmd

B, C, H, W = 32, 64, 32, 32
N = 16
NCORES = 8
NL = N // NCORES           # 2 branches per core
H2 = H // 2                # 16
FB = H2 * W                # 512 free elems per batch per partition
NTOT = float(B * H * W)    # 32768 elements reduced per channel
EPS = 1e-5
F32 = mybir.dt.float32

# Read chunks (in batches): uniform ~1 MiB chunks keep the HWDGE stream
# effectively serial (descriptor generation stays just ahead of the
# drain), which empirically holds the read at line rate; a small final
# chunk keeps the stats tail behind the last DMA short. Bigger, uneven
# chunks make the SDMA engines round-robin across several queues and
# skew chunk completions (measured 11% slower).
# Tail sized by simulation against measured engine rates: ACT sustains
# ~0.61 us/batch (incl. accumulator reads) vs 0.72 us/batch arrival, so
# 2-batch tail chunks minimize the post-read stats drain (~2.1 us);
# 1-batch tails lose (ACT per-chunk fixed cost exceeds its slack).
CHUNKS = [1, 4, 4, 4, 4, 4, 3, 2, 2, 2, 2]
assert sum(CHUNKS) == B

# Output store groups (in batches): ~1 MiB per-branch stores for the
# same reason; ramp up so the first store issues right after the fold.
WGROUPS = [1, 1, 2, 4, 4, 4, 4, 4, 4, 4]
assert sum(WGROUPS) == B
WGMAX = max(WGROUPS)

_NC_CACHE = {}


def _build():
    # Bacc (not raw Bass): its generate_event_semaphores pass legalizes
    # instructions down to <=1 sync-wait each (walrus TS encodings cannot
    # carry more).
    nc = bacc.Bacc("TRN2", num_devices=NCORES, target_bir_lowering=False,
                   debug=False)
    x = nc.dram_tensor("x", [B, C, H, W], F32, kind="ExternalInput")
    gn = nc.dram_tensor("gn", [2 * C, NL], F32, kind="ExternalInput")
    bn = nc.dram_tensor("bn", [2 * C, NL], F32, kind="ExternalInput")
    out = nc.dram_tensor("out", [B, NL * C, H, W], F32, kind="ExternalOutput")

    # [128, 32, 512]: partition (c h0), free (b, h1*w)
    x_re = x.ap().rearrange("b c (h0 h1) w -> (c h0) b (h1 w)", h0=2)
    # [2, 128, 32, 512]: per-branch views, stored separately.
    out_re = out.ap().rearrange("b (n c) (h0 h1) w -> n (c h0) b (h1 w)",
                                n=NL, h0=2)

    nchunk = len(CHUNKS)

    with tile.TileContext(nc) as tc:
        with (
            tc.tile_pool(name="xin", bufs=1) as xin,
            tc.tile_pool(name="consts", bufs=1) as consts,
            tc.tile_pool(name="small", bufs=1) as small,
            tc.tile_pool(name="outs", bufs=12) as outs,
        ):
            sbuf_eps = small.tile([128, 1], F32)
            nc.vector.memset(sbuf_eps, EPS)
            # Dummy Sqrt: forces the single ACT table load (sqrt set,
            # which also contains Square) to happen at kernel start.
            junk1 = small.tile([128, 1], F32, tag="junk1")
            nc.scalar.activation(out=junk1, in_=sbuf_eps,
                                 func=mybir.ActivationFunctionType.Sqrt,
                                 bias=sbuf_eps[:, :])

            # Per-(c,h0) gamma/beta for this core's branches, pre-transposed
            # on host: [128, 2].
            g_sb = consts.tile([2 * C, NL], F32)
            b_sb = consts.tile([2 * C, NL], F32)
            nc.gpsimd.dma_start(out=g_sb, in_=gn.ap())
            nc.gpsimd.dma_start(out=b_sb, in_=bn.ap())

            # Full x, loaded in batch chunks; per chunk two accumulating
            # passes pipeline behind the DMA: DVE reduces -S (negated sum),
            # ACT accumulates Q/N (Square of x*sqrt(1/N)).
            x_sb = xin.tile([2 * C, B, FB], F32)
            junk_q = small.tile([128, max(CHUNKS) * FB], F32, tag="junk_q")
            sq_cols = small.tile([128, 2, nchunk], F32)
            b0 = 0
            for ci, nb in enumerate(CHUNKS):
                # All chunks on ONE HWDGE ring: a second ring would drain
                # concurrently at packet granularity, skewing chunk
                # completion order and stalling the stats pipeline.
                nc.sync.dma_start(out=x_sb[:, b0:b0 + nb, :],
                                  in_=x_re[:, b0:b0 + nb, :])
                xc = x_sb[:, b0:b0 + nb, :].rearrange("p b f -> p (b f)")
                s_col = sq_cols[:, 0, ci:ci + 1].rearrange("p a -> p (a)")
                q_col = sq_cols[:, 1, ci:ci + 1].rearrange("p a -> p (a)")
                nc.vector.tensor_reduce(
                    out=s_col, in_=xc, axis=mybir.AxisListType.X,
                    op=mybir.AluOpType.add, negate=True)
                nc.scalar.activation(
                    out=junk_q[:, 0:nb * FB], in_=xc,
                    func=mybir.ActivationFunctionType.Square,
                    accum_out=q_col)
                b0 += nb

            # (-S, Q) per partition (per H-half), then pair-combine via the
            # DVE 32-way partition permute: swapped[p] = part[p^1].
            part = small.tile([128, 2], F32)
            nc.vector.reduce_sum(out=part, in_=sq_cols,
                                 axis=mybir.AxisListType.X)
            swapped = small.tile([128, 2], F32)
            pairswap = [i ^ 1 for i in range(32)]
            nc.vector.stream_shuffle(out=swapped, in_=part[:, :],
                                     mask=pairswap)
            stt = small.tile([128, 2], F32)  # (-S_tot, Q_tot) per channel
            nc.vector.tensor_add(out=stt, in0=part[:, :], in1=swapped)

            # (nmean, ex2) = stt/NTOT; -var = nmean^2 - ex2;
            # sd = Sqrt(-var * -1 + eps).
            sc2 = small.tile([128, 2], F32)
            nc.vector.tensor_scalar_mul(out=sc2, in0=stt,
                                        scalar1=1.0 / NTOT)
            nmean = sc2[:, 0:1]
            nvar = small.tile([128, 1], F32)
            nc.vector.tensor_scalar(
                out=nvar, in0=nmean, scalar1=sc2[:, 0:1],
                scalar2=sc2[:, 1:2],
                op0=mybir.AluOpType.mult, op1=mybir.AluOpType.subtract)
            sd = small.tile([128, 1], F32)
            nc.scalar.activation(out=sd, in_=nvar,
                                 func=mybir.ActivationFunctionType.Sqrt,
                                 scale=-1.0, bias=sbuf_eps[:, :])
            inv = small.tile([128, 1], F32)
            nc.vector.reciprocal(out=inv, in_=sd)

            # A = gamma*inv ; Bc = beta + nmean*A.
            a_sb = consts.tile([128, NL], F32)
            nc.vector.tensor_scalar_mul(out=a_sb, in0=g_sb, scalar1=inv)
            bc_sb = consts.tile([128, NL], F32)
            nc.vector.scalar_tensor_tensor(
                out=bc_sb, in0=a_sb, scalar=nmean, in1=b_sb,
                op0=mybir.AluOpType.mult, op1=mybir.AluOpType.add)

            # Main loop: fused multiply-add + ~1 MiB store per (branch,
            # group). Branch 0 tiles come from DVE (tensor_scalar), branch
            # 1 tiles from the otherwise-idle ACT engine (Identity with
            # per-partition AP scale/bias computes x*A + Bc too) - the two
            # first tiles are produced in parallel right after the fold,
            # and a DVE p-state throttle can't stall the store stream.
            # Half-batch first stores were tried and reverted: the write
            # stream start is issue-latency-bound, so tiny stores drain
            # dry and delay the steady stream (+0.25 us end-to-end).
            gb0 = 0
            for wg in WGROUPS:
                for j in range(NL):
                    o = outs.tile([128, WGMAX * FB], F32, tag="o")
                    xg = x_sb[:, gb0:gb0 + wg, :].rearrange("p b f -> p (b f)")
                    if j == 0:
                        nc.vector.tensor_scalar(
                            out=o[:, 0:wg * FB], in0=xg,
                            scalar1=a_sb[:, j:j + 1],
                            scalar2=bc_sb[:, j:j + 1],
                            op0=mybir.AluOpType.mult,
                            op1=mybir.AluOpType.add,
                        )
                    else:
                        nc.scalar.activation(
                            out=o[:, 0:wg * FB], in_=xg,
                            func=mybir.ActivationFunctionType.Identity,
                            scale=a_sb[:, j:j + 1], bias=bc_sb[:, j:j + 1],
                        )
                    nc.sync.dma_start(out=out_re[j][:, gb0:gb0 + wg, :],
                                      in_=o[:, 0:wg * FB])
                gb0 += wg
    # Run Bacc's compile pipeline (event-sem legalization, register
    # allocation); the PJRT execute path serializes without finalizing.
    nc.finalize()
    return nc


def _get_nc():
    if "nc" not in _NC_CACHE:
        _NC_CACHE["nc"] = _build()
    return _NC_CACHE["nc"]


def _run(inputs, **kwargs):
    x = np.ascontiguousarray(np.asarray(inputs["x"], dtype=np.float32))
    gamma = np.asarray(inputs["gamma"], dtype=np.float32)
    beta = np.asarray(inputs["beta"], dtype=np.float32)
    g128 = np.ascontiguousarray(np.repeat(gamma.T, 2, axis=0))  # [128, 16]
    b128 = np.ascontiguousarray(np.repeat(beta.T, 2, axis=0))
    in_maps = [
        {"x": x,
         "gn": np.ascontiguousarray(g128[:, i * NL:(i + 1) * NL]),
         "bn": np.ascontiguousarray(b128[:, i * NL:(i + 1) * NL])}
        for i in range(NCORES)
    ]
    nc = _get_nc()
    res = run_bass_kernel_spmd(nc, in_maps, core_ids=list(range(NCORES)), **kwargs)
    # Core i computed branches [i*NL, (i+1)*NL) -> channel block of NL*C.
    full = np.concatenate([r["out"] for r in res.results], axis=1)
    return full, res


def kernel(**inputs):
    full, _ = _run(inputs)
    return full
